# revision 1
# baseline (speedup 1.0000x reference)
"""SPMD Trainium2 kernel for nn_CombinedGraphLayer_67791763800346.

kernel(**inputs): FULL unsharded inputs (as in reference.setup_inputs()),
returns FULL output [8, 12800, 256] f32. Data-parallel over the batch axis:
one NeuronCore per batch element, no collectives. Self-contained: the
builder (kb.py) is inlined below.
"""
import os
import sys
sys.path.insert(0, "/opt/trn_rl_repo")

from contextlib import ExitStack

import numpy as np

import concourse.bacc as bacc
import concourse.tile as tile
from concourse import bass_utils

# ======================= inlined builder (kb) =======================
"""Bass/Tile kernel builder for nn_CombinedGraphLayer (LSH-binned graph conv).

Per-core problem (one batch element):
  x [N, 256] f32 -> LayerNorm -> 3-layer MLP (elu) -> x_dist [N, 128]
  -> LSH proj [N, nb/2] -> argmax over [proj, -proj] (nb bins)
  -> stable counting sort -> nb bins x 128 points
  -> per-bin pairwise gaussian kernel + gated graph conv -> scatter back.

Precision strategy (validated numerically):
  - LN, MLP, proj, argmax: fp32 (bf16 causes ~1% bin flips -> 10% rel err)
  - everything post-binning: bf16 (measured ~0.4% rel err)

Permutation machinery (HW-validated): indirect_dma_start with [128,1]
offsets; gather sources must be Internal DRAM, scatters work to any DRAM.

Layouts: "pm" = points-major [128 pts, F]; "fm" = feature-major [F-chunk, pts].
"""

from contextlib import ExitStack

import numpy as np

import concourse.bass as bass
import concourse.tile as tile
from concourse.tile import add_dep_helper
from concourse import mybir
import concourse.hw_specs as _hw_specs

# Steer the ACT table-set chooser: keep our hot functions (Ln/Exp/Relu/
# Identity/Copy) resolvable only via natural_log_exp_and_others so every
# activation in the kernel shares one table set (Sigmoid stays in its own
# set, used once per supergroup). Runtime behavior is unchanged -- walrus
# loads the real act_info sets; this only affects which set id gets picked.
_orig_get_tables = _hw_specs.get_activation_tables


def _steered_tables(module_arch):
    tabs = _orig_get_tables(module_arch)
    A = mybir.ActivationFunctionType
    hot = {A.Ln, A.Exp, A.Relu, A.Identity, A.Copy}
    out = {}
    for name, funcs in tabs.items():
        if name == "natural_log_exp_and_others":
            out[name] = funcs
        else:
            out[name] = funcs - hot
    return out


STEER_TABLES = True
import concourse.bacc as _bacc_mod
if STEER_TABLES:
    _hw_specs.get_activation_tables = _steered_tables
    _bacc_mod.get_activation_tables = _steered_tables

F32 = mybir.dt.float32
BF16 = mybir.dt.bfloat16
I32 = mybir.dt.int32
I16 = mybir.dt.int16
AF = mybir.ActivationFunctionType
ALU = mybir.AluOpType
AX = mybir.AxisListType
P = 128


def host_inputs(N, weights):
    """Per-core input dict (everything except 'x'), gamma/beta folded."""
    nb = N // P
    g = weights["ln_gamma"].astype(np.float32)
    be = weights["ln_beta"].astype(np.float32)
    w0 = g[:, None] * weights["w0"].astype(np.float32)
    b0 = weights["b0"].astype(np.float32) + be @ weights["w0"].astype(np.float32)
    th = g[:, None] * weights["theta"].astype(np.float32)
    wh = g[:, None] * weights["W_h"].astype(np.float32)
    wt = g[:, None] * weights["W_t"].astype(np.float32)
    assert np.abs(be).max() == 0.0, "nonzero ln_beta not supported in post-bin path"
    assert np.abs(weights["b_t"]).max() == 0.0, "nonzero b_t not supported"
    bf = lambda a: np.ascontiguousarray(a, np.float32)
    return {
        "w0": bf(w0), "w1": bf(weights["w1"]), "w2": bf(weights["w2"]),
        "rot": bf(weights["rotations"][:, : nb // 2]),
        "theta": bf(th), "wh": bf(wh), "whn": bf(-wh), "wt": bf(wt),
        "b0": bf(b0), "b1": bf(weights["b1"]), "b2": bf(weights["b2"]),
        "tri_incl": np.triu(np.ones((P, P), np.float32)),
        "su": np.triu(np.ones((P, P), np.float32), 1),
        "ident": np.eye(P, dtype=np.float32),
        "rev_row": (nb - np.arange(nb, dtype=np.float32))[None, :],
        "ones_col": np.ones((P, 1), np.float32),
    }


def declare_io(nc, N):
    nb = N // P
    t = {}

    def inp(name, shape, dt=F32):
        t[name] = nc.dram_tensor(name, shape, dt, kind="ExternalInput").ap()

    inp("x", [N, 256])
    inp("w0", [256, 256]); inp("w1", [256, 256]); inp("w2", [256, 128])
    inp("rot", [128, nb // 2])
    inp("theta", [256, 256]); inp("wh", [256, 256]); inp("whn", [256, 256])
    inp("wt", [256, 256])
    inp("b0", [256]); inp("b1", [256]); inp("b2", [128])
    inp("tri_incl", [P, P]); inp("su", [P, P]); inp("ident", [P, P])
    inp("rev_row", [1, nb]); inp("ones_col", [P, 1])

    t["out"] = nc.dram_tensor("out", [N, 256], F32, kind="ExternalOutput").ap()
    return t


def _bcast_mid(ap2d, mid, last):
    return ap2d.rearrange("p (c x) -> p c x", c=1).to_broadcast(
        [ap2d.shape[0], mid, last])


def emit(ctx: ExitStack, tc: tile.TileContext, t, N, stage="full"):
    nc = tc.nc
    def _raw(inst):
        return getattr(inst, "ins", inst)

    _act_prev = [None, False, None]  # [last_main, in_phase_b, last_sig]
    _real_scalar = tc.nc.scalar
    _orig_act = _real_scalar.activation

    def _act_chained(*a, **k):
        inst = _orig_act(*a, **k)
        if _act_prev[1]:
            raw = _raw(inst)
            if k.get("func") == AF.Sigmoid:
                if _act_prev[2] is not None:
                    add_dep_helper(raw, _act_prev[2], sync=False, reason="sig chain")
                if _act_prev[0] is not None:
                    add_dep_helper(raw, _act_prev[0], sync=False, reason="sig fence")
                _act_prev[2] = raw
            else:
                if _act_prev[2] is not None:
                    add_dep_helper(raw, _act_prev[2], sync=False, reason="main fence")
                _act_prev[0] = raw
        return inst

    class _ScalarShim:
        def __getattr__(self, nm):
            return getattr(_real_scalar, nm)

    _shim = _ScalarShim()
    _shim.__dict__["activation"] = _act_chained
    _real_nc = tc.nc

    class _NCShim:
        def __getattr__(self, nm):
            if nm == "scalar":
                return _shim
            return getattr(_real_nc, nm)

    nc = _NCShim()

    def _fence(inst, dep):
        return inst
    nb = N // P               # bins == tiles of 128 points
    nc4 = N // 512            # phase-A chunks
    assert N % 512 == 0 and nb % 4 == 0
    nhalf = nb // 2
    nq = nb // 4

    t = dict(t)
    for name, shape, dt in [
        ("xnxd", [N, 384], BF16),          # cols 0:256 xn, 256:384 xd
        ("res_d", [N, 256], BF16),
        ("flat0", [N, 1], I32), ("flat1", [N, 1], I32),
        ("flat2", [N, 1], I32), ("flat3", [N, 1], I32),
        ("c_d", [1, nb * nb], I16),
        ("g_d", [1, nb], F32),
    ]:
        t[name] = nc.dram_tensor(name, shape, dt, kind="Internal").ap()

    consts = ctx.enter_context(tc.tile_pool(name="consts", bufs=1))
    persist = ctx.enter_context(tc.tile_pool(name="persist", bufs=1))

    # ---------------- constants ----------------
    def load_const(name, shape, dt=F32):
        tl = consts.tile(shape, dt, tag="c_" + name)
        nc.sync.dma_start(out=tl, in_=t[name])
        return tl

    tri_incl = load_const("tri_incl", [P, P])
    su_f = load_const("su", [P, P])
    ident = load_const("ident", [P, P])
    ones_col = load_const("ones_col", [P, 1])
    rot_s = load_const("rot", [P, nb // 2])
    ident_bf = consts.tile([P, P], BF16)
    nc.vector.tensor_copy(out=ident_bf, in_=ident)
    tri_bf = consts.tile([P, P], BF16)
    nc.vector.tensor_copy(out=tri_bf, in_=tri_incl)
    ones_bf = consts.tile([P, 1], BF16)
    nc.vector.tensor_copy(out=ones_bf, in_=ones_col)

    def load_w(name, out_dim, dt):
        tl = consts.tile([P, 2, out_dim], dt, tag="w_" + name)
        if dt == F32:
            nc.sync.dma_start(out=tl, in_=t[name].rearrange("(c p) o -> p c o", p=P))
        else:
            nc.gpsimd.dma_start(out=tl, in_=t[name].rearrange("(c p) o -> p c o", p=P))
        return tl

    w0_s = load_w("w0", 256, F32)
    w1_s = load_w("w1", 256, F32)
    w2_s = load_w("w2", 128, F32)
    th_s = load_w("theta", 256, BF16)
    wh_s = load_w("wh", 256, BF16)
    whn_s = load_w("whn", 256, BF16)
    wt_s = load_w("wt", 256, BF16)

    def load_b(name, chunks):
        tl = consts.tile([P, chunks], F32, tag="b_" + name)
        nc.sync.dma_start(out=tl, in_=t[name].rearrange("(c p) -> p c", p=P))
        return tl

    b0_s = load_b("b0", 2)
    b1_s = load_b("b1", 2)
    b2_s = load_b("b2", 1)

    rev_t = consts.tile([P, nb], F32)
    rsrc = t["rev_row"]
    nc.sync.dma_start(out=rev_t, in_=bass.AP(tensor=rsrc.tensor, offset=rsrc.offset,
                                             ap=[[0, P], rsrc.ap[1]]))
    eps_col = consts.tile([P, 1], F32)
    nc.vector.memset(eps_col, 1e-6)

    # persistent cross-phase tensors
    o_all = persist.tile([P, nb, nb], BF16)
    v1_all = persist.tile([P, nb], F32)
    v23_all = persist.tile([P, nb], F32)
    pos_f = persist.tile([P, nb], F32)
    pos_i = persist.tile([P, nb], I32)
    fl_i = persist.tile([P, nb], I32)
    c_sb = persist.tile([1, nb * nb], I16)
    r_sb = persist.tile([1, nb], F32)

    # =====================================================================
    # PHASE A
    # =====================================================================
    with tc.tile_pool(name="psA", bufs=1, space="PSUM") as psA, \
         tc.tile_pool(name="psA2", bufs=2, space="PSUM") as psA2, \
         tc.tile_pool(name="sbA", bufs=2) as sbA, \
         tc.tile_pool(name="sbA3", bufs=3) as sbA3:

        cs_prev = None

        def prep_chunk(c):
            r0 = c * 512
            x4 = sbA3.tile([P, 4, 256], F32, tag="x4")
            nc.sync.dma_start(out=x4,
                              in_=t["x"][r0:r0 + 512, :].rearrange("(a p) f -> p a f", p=P))

            # --- LayerNorm ---
            st = sbA.tile([P, 4, 6], F32, tag="st")
            mv = sbA.tile([P, 4, 2], F32, tag="mv")
            for i in range(4):
                nc.vector.bn_stats(out=st[:, i, :], in_=x4[:, i, :])
                nc.vector.bn_aggr(out=mv[:, i, :], in_=st[:, i, :])
            rstd = sbA.tile([P, 4], F32, tag="rstd")
            nmu = sbA.tile([P, 4], F32, tag="nmu")
            lv = sbA.tile([P, 4], F32, tag="lv")
            nc.scalar.activation(out=lv, in_=mv[:, :, 1], func=AF.Ln, bias=eps_col)
            nc.scalar.activation(out=rstd, in_=lv, func=AF.Exp, scale=-0.5)
            nc.vector.tensor_mul(nmu, mv[:, :, 0], rstd)
            nc.vector.tensor_scalar_mul(nmu, nmu, -1.0)

            xn32 = sbA3.tile([P, 4, 256], F32, tag="xn32")
            for i in range(4):
                nc.vector.tensor_scalar(out=xn32[:, i, :], in0=x4[:, i, :],
                                        scalar1=rstd[:, i:i + 1], scalar2=nmu[:, i:i + 1],
                                        op0=ALU.mult, op1=ALU.add)
            xnb = sbA.tile([P, 4, 256], BF16, tag="xnb")
            nc.gpsimd.tensor_copy(out=xnb, in_=xn32)
            nc.sync.dma_start(
                out=t["xnxd"][r0:r0 + 512, 0:256].rearrange("(a p) f -> p a f", p=P),
                in_=xnb)

            if stage == "ln":
                nc.sync.dma_start(
                    out=t["out"][r0:r0 + 512, :].rearrange("(a p) f -> p a f", p=P),
                    in_=xn32)
                return None

            # --- transpose to feature-major (part of prep) ---
            xnT_ps = psA.tile([P, 8, P], F32, tag="xnT")
            for i in range(4):
                for fi in range(2):
                    nc.tensor.transpose(out=xnT_ps[:, i * 2 + fi, :],
                                        in_=xn32[:, i, fi * P:(fi + 1) * P],
                                        identity=ident)
            xn_fm = sbA.tile([P, 8, P], F32, tag="xn_fm")
            nc.scalar.copy(out=xn_fm, in_=xnT_ps)
            return xn_fm

        prepped = prep_chunk(0)
        for c in range(nc4):
            xn_fm = prepped
            if c + 1 < nc4:
                prepped = prep_chunk(c + 1)
            if xn_fm is None:
                continue
            r0 = c * 512
            xn_fm_v = xn_fm[:].rearrange("p (a f) x -> p a f x", f=2)
            # --- MLP ---
            def mlp_layer(rhs_of_fi, w_s, b_s, tag, act_form):
                hp = psA2.tile([P, 1024], F32, tag="h_ps")
                for fo in range(2):
                    for fi in range(2):
                        nc.tensor.matmul(out=hp[:, fo * 512:(fo + 1) * 512],
                                         lhsT=w_s[:, fi, fo * P:(fo + 1) * P],
                                         rhs=rhs_of_fi(fi),
                                         start=(fi == 0), stop=(fi == 1))
                hb = sbA.tile([P, 1024], F32, tag=tag + "_hb")
                tmin = sbA.tile([P, 1024], F32, tag=tag + "_t")
                for fo in range(2):
                    sl = slice(fo * 512, (fo + 1) * 512)
                    nc.scalar.activation(out=hb[:, sl], in_=hp[:, sl], func=AF.Identity,
                                         bias=b_s[:, fo:fo + 1])
                if act_form:
                    nc.scalar.activation(out=tmin, in_=hb, func=AF.Relu, scale=-1.0)
                    e = sbA.tile([P, 1024], F32, tag=tag + "_e")
                    nc.scalar.activation(out=e, in_=tmin, func=AF.Exp, scale=-1.0)
                else:
                    nc.vector.tensor_scalar_min(tmin, hb, 0.0)
                    e = sbA.tile([P, 1024], F32, tag=tag + "_e")
                    nc.scalar.activation(out=e, in_=tmin, func=AF.Exp)
                m = sbA.tile([P, 1024], F32, tag=tag + "_m")
                nc.gpsimd.tensor_scalar_add(m, e, -1.0)
                hs = sbA.tile([P, 1024], F32, tag=tag)
                nc.vector.tensor_tensor(out=hs, in0=hb, in1=m, op=ALU.max)
                return hs

            h1 = mlp_layer(lambda fi: xn_fm_v[:, :, fi, :], w0_s, b0_s, "h1", True)
            h2 = mlp_layer(lambda fi: h1[:, fi * 512:(fi + 1) * 512], w1_s, b1_s,
                           "h2", False)

            if stage == "mlp":
                nc.sync.dma_start(
                    out=t["out"][r0:r0 + 512, :].rearrange("(a p) f -> p a f", p=P),
                    in_=h2[:].rearrange("p (a f) -> p a f", a=4))
                continue
            # --- x_dist ---
            xdp = psA.tile([P, 512], F32, tag="xd")
            for fi in range(2):
                nc.tensor.matmul(out=xdp, lhsT=w2_s[:, fi, :],
                                 rhs=h2[:, fi * 512:(fi + 1) * 512],
                                 start=(fi == 0), stop=(fi == 1))
            xd32 = sbA.tile([P, 512], F32, tag="xd32")
            nc.scalar.activation(out=xd32, in_=xdp, func=AF.Identity, bias=b2_s[:, 0:1])

            xdT_ps = psA.tile([P, 8, P], F32, tag="xnT")
            for i in range(4):
                nc.tensor.transpose(out=xdT_ps[:, i, :], in_=xd32[:, i * P:(i + 1) * P],
                                    identity=ident)
            xd_pm = sbA.tile([P, 4, P], BF16, tag="xd_pm")
            nc.scalar.copy(out=xd_pm, in_=xdT_ps[:, 0:4, :])
            nc.sync.dma_start(
                out=t["xnxd"][r0:r0 + 512, 256:384].rearrange("(a p) f -> p a f", p=P),
                in_=xd_pm)

            if stage in ("a", "xd", "proj"):
                nc.sync.dma_start(
                    out=t["out"][r0:r0 + 512, :].rearrange("(a p) f -> p a f", p=P),
                    in_=xn32)
            if stage == "xd":
                continue
            # --- proj / argmax / one-hot ---
            pj = psA.tile([P, 4 * (nb // 2) + 2 * nb], F32, tag="pj")
            projp = pj[:, 0:4 * (nb // 2)].rearrange("p (a r) -> p a r", a=4)
            pp = pj[:, 4 * (nb // 2):4 * (nb // 2) + nb]
            cs_region = pj[0:1, 4 * (nb // 2) + nb:]
            for i in range(4):
                nc.tensor.matmul(out=projp[:, i, :], lhsT=xd32[:, i * P:(i + 1) * P],
                                 rhs=rot_s, start=True, stop=True)
            cmul = sbA.tile([P, 4, nb], F32, tag="cmul")
            nc.vector.tensor_copy(out=cmul[:, :, 0:nb // 2], in_=projp)
            nc.scalar.activation(out=cmul[:, :, nb // 2:nb], in_=projp,
                                 func=AF.Copy, scale=-1.0)
            mx = sbA.tile([P, 4], F32, tag="mx")
            nc.vector.reduce_max(out=mx, in_=cmul, axis=AX.X)
            eq = sbA.tile([P, 4, nb], F32, tag="eq")
            for i in range(4):
                nc.vector.tensor_scalar(out=eq[:, i, :], in0=cmul[:, i, :],
                                        scalar1=mx[:, i:i + 1], scalar2=None,
                                        op0=ALU.is_ge)
            score = sbA.tile([P, 4, nb], F32, tag="score")
            nc.vector.tensor_tensor(out=score, in0=eq, in1=_bcast_mid(rev_t[:], 4, nb),
                                    op=ALU.mult)
            smax = sbA.tile([P, 4], F32, tag="smax")
            nc.vector.reduce_max(out=smax, in_=score, axis=AX.X)
            for i in range(4):
                g = c * 4 + i
                nc.vector.tensor_scalar(out=o_all[:, g, :], in0=score[:, i, :],
                                        scalar1=smax[:, i:i + 1], scalar2=None,
                                        op0=ALU.is_equal)

            if stage == "proj":
                continue
            # --- sort bookkeeping ---
            for i in range(4):
                g = c * 4 + i
                if stage != "sk1":
                    if g == 0:
                        nc.vector.memset(c_sb[:, 0:nb], 0)
                    else:
                        nc.vector.tensor_tensor(out=c_sb[:, g * nb:(g + 1) * nb],
                                                in0=c_sb[:, (g - 1) * nb:g * nb],
                                                in1=cs_prev, op=ALU.add)
                    cs = cs_region
                    nc.tensor.matmul(out=cs, lhsT=ones_bf, rhs=o_all[:, g, :],
                                     start=True, stop=True)
                    cs_prev = cs
                if stage != "sk2":
                    nc.tensor.matmul(out=pp, lhsT=tri_bf, rhs=o_all[:, g, :],
                                     start=True, stop=True)
                    prod = sbA.tile([P, nb], F32, tag="prod")
                    nc.vector.tensor_tensor(out=prod, in0=pp,
                                            in1=o_all[:, g, :], op=ALU.mult)
                    nc.vector.reduce_sum(out=v1_all[:, g:g + 1],
                                         in_=prod[:].rearrange("p (a x) -> p a x", a=1),
                                         axis=AX.X)

        if cs_prev is not None:
            nc.vector.tensor_tensor(out=r_sb, in0=c_sb[:, (nb - 1) * nb:nb * nb],
                                    in1=cs_prev, op=ALU.add)

    if stage in ("a", "ln", "mlp", "xd", "proj", "sk1", "sk2", "sk1a", "sk1b"):
        return
    # =====================================================================
    # SORT FINALIZATION -> pos, flat
    # =====================================================================
    with tc.tile_pool(name="psS", bufs=1, space="PSUM") as psS, \
         tc.tile_pool(name="sbS", bufs=2) as sbS:
        totT_ps = psS.tile([nb, 1], F32, tag="totT")
        nc.tensor.transpose(out=totT_ps, in_=r_sb, identity=ident[0:1, 0:1])
        tot_col = sbS.tile([nb, 1], F32, tag="tot_col")
        nc.vector.tensor_copy(out=tot_col, in_=totT_ps)
        gb_ps = psS.tile([nb, 1], F32, tag="gb_ps")
        nc.tensor.matmul(out=gb_ps, lhsT=su_f[0:nb, 0:nb], rhs=tot_col,
                         start=True, stop=True)
        gb_col = sbS.tile([nb, 1], F32, tag="gb_col")
        nc.vector.tensor_scalar_add(gb_col, gb_ps, -1.0)   # gbase - 1
        gbT_ps = psS.tile([1, nb], F32, tag="gbT")
        nc.tensor.transpose(out=gbT_ps, in_=gb_col, identity=ident[0:nb, 0:nb])
        gm1 = sbS.tile([1, nb], F32, tag="gm1")
        nc.vector.tensor_copy(out=gm1, in_=gbT_ps)
        nc.sync.dma_start(out=t["g_d"], in_=gm1)
        nc.sync.dma_start(out=t["c_d"], in_=c_sb)

        gb_t = sbS.tile([P, nb], F32, tag="gb_t")
        gsrc = t["g_d"]
        nc.sync.dma_start(out=gb_t, in_=bass.AP(tensor=gsrc.tensor, offset=gsrc.offset,
                                                ap=[[0, P], gsrc.ap[1]]))
        # flat machinery set up first so per-q-chunk scatters can start as
        # soon as that chunk's positions are known (overlaps v23 extraction).
        iota_pl = sbS.tile([P, nb], I32, tag="iota_pl")
        nc.gpsimd.iota(out=iota_pl, pattern=[[P, nb]], base=1, channel_multiplier=1)
        zt32 = sbS.tile([P, N // P], I32, tag="zt32")
        nc.vector.memset(zt32, 0)
        flats = [t["flat0"], t["flat1"], t["flat2"], t["flat3"]]
        for fk in flats:
            nc.sync.dma_start(
                out=fk.rearrange("(p a) x -> p (a x)", p=P),
                in_=zt32[:])
        for q in range(4):
            cb = sbS.tile([P, nq, nb], I16, tag="cb")
            csrc = t["c_d"]
            nc.sync.dma_start(out=cb, in_=bass.AP(
                tensor=csrc.tensor, offset=csrc.offset + q * nq * nb,
                ap=[[0, P], [nb, nq], [1, nb]]))
            cbp = sbS.tile([P, nq, nb], F32, tag="cbp")
            nc.vector.tensor_tensor(out=cbp, in0=cb, in1=_bcast_mid(gb_t[:], nq, nb),
                                    op=ALU.add)
            w = sbS.tile([P, nq, nb], F32, tag="wscr")
            nc.vector.tensor_tensor(out=w, in0=o_all[:, q * nq:(q + 1) * nq, :],
                                    in1=cbp, op=ALU.mult)
            ql, qh = q * nq, (q + 1) * nq
            nc.vector.reduce_sum(out=v23_all[:, ql:qh], in_=w, axis=AX.X)
            nc.vector.tensor_add(pos_f[:, ql:qh], v1_all[:, ql:qh],
                                 v23_all[:, ql:qh])
            nc.vector.tensor_copy(out=pos_i[:, ql:qh], in_=pos_f[:, ql:qh])
            for g in range(ql, qh):
                nc.gpsimd.indirect_dma_start(
                    out=flats[g % 4],
                    out_offset=bass.IndirectOffsetOnAxis(ap=pos_i[:, g:g + 1], axis=0),
                    in_=iota_pl[:, g:g + 1], in_offset=None)
        flk = sbS.tile([P, 4, nb], I32, tag="flk")
        for k, fk in enumerate(flats):
            nc.sync.dma_start(out=flk[:, k, :], in_=bass.AP(
                tensor=fk.tensor, offset=fk.offset, ap=[[1, P], [P, nb]]))
        m01 = sbS.tile([P, nb], I32, tag="m01")
        nc.vector.tensor_tensor(out=m01, in0=flk[:, 0, :], in1=flk[:, 1, :], op=ALU.max)
        m23 = sbS.tile([P, nb], I32, tag="m23")
        nc.vector.tensor_tensor(out=m23, in0=flk[:, 2, :], in1=flk[:, 3, :], op=ALU.max)
        mall = sbS.tile([P, nb], I32, tag="mall")
        nc.vector.tensor_tensor(out=mall, in0=m01, in1=m23, op=ALU.max)
        nc.vector.tensor_scalar_add(fl_i, mall, -1.0)

        if stage == "sort":
            # debug: dump flat and pos into the first out rows
            dbg = sbS.tile([P, 2, nb], F32, tag="dbg")
            nc.vector.tensor_copy(out=dbg[:, 0, :], in_=fl_i)
            nc.vector.tensor_copy(out=dbg[:, 1, :], in_=pos_i)
            nc.sync.dma_start(out=t["out"][0:P, 0:2 * nb], in_=dbg)
    if stage == "sort":
        return
    # =====================================================================
    # PHASE B: per 2-bin group; results scattered straight to `out`
    # =====================================================================
    for cand in (20, 16, 10, 8, 4, 2):
        if nb % cand == 0:
            SG = cand
            break
    with tc.tile_pool(name="psB", bufs=1, space="PSUM") as psB, \
         tc.tile_pool(name="psB2", bufs=2, space="PSUM") as psB2, \
         tc.tile_pool(name="sbB", bufs=3) as sbB, \
         tc.tile_pool(name="sbSG", bufs=2) as sbSG:
        prev_main_act = None
        last_sig = None
        _act_prev[1] = True
        for sgi in range(nb // SG):
            b0 = sgi * SG
            gx = sbSG.tile([P, SG, 384], BF16, tag="gx")
            for i in range(SG):
                nc.gpsimd.indirect_dma_start(
                    out=gx[:, i, :], out_offset=None, in_=t["xnxd"],
                    in_offset=bass.IndirectOffsetOnAxis(
                        ap=fl_i[:, b0 + i:b0 + i + 1], axis=0))
            xng = gx[:, :, 0:256]
            xdg = gx[:, :, 256:384]

            # transposes for the whole supergroup
            xdT = sbSG.tile([P, SG, P], BF16, tag="xdT")
            xfT = sbSG.tile([P, SG, 2, P], BF16, tag="xfT")
            for m in range(SG // 2):
                pw1 = psB.tile([P, 6, P], BF16, tag="pw1")
                for i in range(2):
                    bi = 2 * m + i
                    nc.tensor.transpose(out=pw1[:, i, :], in_=xdg[:, bi, :],
                                        identity=ident_bf)
                    for hh in range(2):
                        nc.tensor.transpose(out=pw1[:, 2 + i * 2 + hh, :],
                                            in_=xng[:, bi, hh * P:(hh + 1) * P],
                                            identity=ident_bf)
                _shim.activation(out=xdT[:, 2 * m:2 * m + 2, :], in_=pw1[:, 0:2, :], func=AF.Copy)
                _shim.activation(out=xfT[:, 2 * m:2 * m + 2, :, :].rearrange(
                    "p a b x -> p (a b) x"), in_=pw1[:, 2:6, :], func=AF.Copy)

            # gate superstep (sigmoid table set used once per supergroup)
            gts = sbSG.tile([P, SG, 256], BF16, tag="gts")
            for m in range(SG // 2):
                gate_ps = psB2.tile([P, 2, 256], F32, tag="gate_ps")
                for i in range(2):
                    bi = 2 * m + i
                    for ko in range(2):
                        nc.tensor.matmul(out=gate_ps[:, i, :],
                                         lhsT=xfT[:, bi, ko, :],
                                         rhs=wt_s[:, ko, :],
                                         start=(ko == 0), stop=(ko == 1))
                sig = nc.scalar.activation(out=gts[:, 2 * m:2 * m + 2, :], in_=gate_ps,
                                           func=AF.Sigmoid)
                _fence(sig, prev_main_act)
                last_sig = sig

            # main superstep (ln/exp table set)
            for m in range(SG // 2):
                na = sbB.tile([P, 2], F32, tag="na")
                nascr = sbB.tile([P, 2, P], F32, tag="nascr")
                nc.vector.tensor_tensor(out=nascr, in0=xdg[:, 2 * m:2 * m + 2, :],
                                        in1=xdg[:, 2 * m:2 * m + 2, :], op=ALU.mult)
                nc.vector.reduce_sum(out=na, in_=nascr, axis=AX.X)

                pw2a = psB.tile([P, 4, P], F32, tag="pw2a")
                for i in range(2):
                    bi = 2 * m + i
                    nc.tensor.matmul(out=pw2a[:, i, :], lhsT=xdT[:, bi, :],
                                     rhs=xdT[:, bi, :], start=True, stop=True)
                a_s = sbB.tile([P, 2, P], F32, tag="a_s")
                na_b = na[:].rearrange("p (c x) -> p x c", c=1).to_broadcast([P, 2, P])
                nc.vector.tensor_tensor(out=a_s, in0=na_b, in1=pw2a[:, 0:2, :],
                                        op=ALU.subtract)
                for i in range(2):
                    nc.tensor.transpose(out=pw2a[:, 2 + i, :], in_=a_s[:, i, :],
                                        identity=ident)
                d2 = sbB.tile([P, 2, P], F32, tag="d2")
                nc.vector.tensor_tensor(out=d2, in0=a_s, in1=pw2a[:, 2:4, :], op=ALU.add)
                nc.vector.tensor_scalar_max(d2, d2, 1e-6)
                lg = sbB.tile([P, 2, P], F32, tag="lg")
                nc.scalar.activation(out=lg, in_=d2, func=AF.Ln)
                sq = sbB.tile([P, 2, P], F32, tag="sq")
                nc.scalar.activation(out=sq, in_=lg, func=AF.Exp, scale=0.5)
                dm = sbB.tile([P, 2, P], BF16, tag="dm")
                nc.scalar.activation(out=dm, in_=sq, func=AF.Exp, scale=-0.1)

                indeg = sbB.tile([P, 2], F32, tag="indeg")
                nc.vector.reduce_sum(out=indeg, in_=dm, axis=AX.X)
                li = sbB.tile([P, 2], F32, tag="li")
                nc.scalar.activation(out=li, in_=indeg, func=AF.Ln, bias=eps_col)
                nrm = sbB.tile([P, 2], F32, tag="nrm")
                nc.scalar.activation(out=nrm, in_=li, func=AF.Exp, scale=-0.5)
                pw2b = psB.tile([P, 4, P], F32, tag="pw2b")
                for i in range(2):
                    nc.tensor.transpose(out=pw2b[0:1, 2 + i, :], in_=nrm[:, i:i + 1],
                                        identity=ident)
                nT = sbB.tile([1, 2, P], BF16, tag="nT")
                _shim.activation(out=nT, in_=pw2b[0:1, 2:4, :], func=AF.Copy)
                for i in range(2):
                    nc.tensor.matmul(out=pw2b[:, i, :], lhsT=nT[0:1, i, :],
                                     rhs=nT[0:1, i, :], start=True, stop=True)
                dmw = sbB.tile([P, 2, P], BF16, tag="dmw")
                nc.vector.tensor_tensor(out=dmw, in0=dm, in1=pw2b[:, 0:2, :], op=ALU.mult)

                t1_ps = psB.tile([P, 2, 256], F32, tag="t1_ps")
                for i in range(2):
                    bi = 2 * m + i
                    for ko in range(2):
                        nc.tensor.matmul(out=t1_ps[:, i, :], lhsT=xfT[:, bi, ko, :],
                                         rhs=th_s[:, ko, :], start=(ko == 0),
                                         stop=(ko == 1))
                t1 = sbB.tile([P, 2, 256], BF16, tag="t1")
                _shim.activation(out=t1, in_=t1_ps, func=AF.Copy)

                hd_ps = psB.tile([P, 2, 256], F32, tag="hd_ps")
                fhet_ps = psB.tile([P, 2, 256], F32, tag="fhet_ps")
                for i in range(2):
                    bi = 2 * m + i
                    nc.tensor.matmul(out=hd_ps[:, i, :], lhsT=dmw[:, i, :],
                                     rhs=t1[:, i, :], start=True, stop=False)
                    for ko in range(2):
                        nc.tensor.matmul(out=hd_ps[:, i, :], lhsT=xfT[:, bi, ko, :],
                                         rhs=whn_s[:, ko, :], start=False,
                                         stop=(ko == 1))
                        nc.tensor.matmul(out=fhet_ps[:, i, :], lhsT=xfT[:, bi, ko, :],
                                         rhs=wh_s[:, ko, :], start=(ko == 0),
                                         stop=(ko == 1))
                tmp = sbB.tile([P, 2, 256], BF16, tag="tmp")
                nc.vector.tensor_tensor(out=tmp, in0=gts[:, 2 * m:2 * m + 2, :],
                                        in1=hd_ps, op=ALU.mult)
                pre = sbB.tile([P, 2, 256], BF16, tag="pre")
                nc.vector.tensor_tensor(out=pre, in0=tmp, in1=fhet_ps, op=ALU.add)

                tmin = sbB.tile([P, 2, 256], BF16, tag="btmin")
                nc.vector.tensor_scalar_min(tmin, pre, 0.0)
                e = sbB.tile([P, 2, 256], BF16, tag="be")
                prev_main_act = nc.scalar.activation(out=e, in_=tmin, func=AF.Exp)
                mm_ = sbB.tile([P, 2, 256], BF16, tag="bm")
                nc.vector.tensor_scalar_add(mm_, e, -1.0)
                res = sbB.tile([P, 2, 256], F32, tag="bres")
                nc.vector.tensor_tensor(out=res, in0=pre, in1=mm_, op=ALU.max)
                for i in range(2):
                    nc.gpsimd.indirect_dma_start(
                        out=t["out"],
                        out_offset=bass.IndirectOffsetOnAxis(
                            ap=fl_i[:, b0 + 2 * m + i:b0 + 2 * m + i + 1], axis=0),
                        in_=res[:, i, :], in_offset=None)

# ======================= SPMD wrapper =======================
N = 12800
_NC_CACHE = {}
LAST_RESULT = None
TRACE = bool(int(os.environ.get("KERNEL_TRACE", "0")))


def _build():
    if N in _NC_CACHE:
        return _NC_CACHE[N]
    nc = bacc.Bacc("TRN2", debug=False)
    t = declare_io(nc, N)
    with tile.TileContext(nc) as tc:
        with ExitStack() as ctx:
            emit(ctx, tc, t, N)
    nc.compile()
    _NC_CACHE[N] = nc
    return nc


def kernel(**inputs):
    global LAST_RESULT
    x = np.asarray(inputs["x"], dtype=np.float32)
    B = x.shape[0]
    assert x.shape == (B, N, 256)
    msk = np.asarray(inputs["msk"])
    assert msk.all(), "kernel assumes msk all ones (spec fill=ones)"

    weights = {k: np.asarray(v) for k, v in inputs.items() if k not in ("x", "msk")}
    shared = host_inputs(N, weights)
    nc = _build()
    in_maps = [{"x": np.ascontiguousarray(x[b]), **shared} for b in range(B)]
    res = bass_utils.run_bass_kernel_spmd(nc, in_maps, core_ids=list(range(B)),
                                          trace=TRACE)
    LAST_RESULT = res
    return np.stack([res.results[b]["out"] for b in range(B)])

